# revision 21
# baseline (speedup 1.0000x reference)
"""Trainium2 Bass kernel for LGAttention (global MHA + windowed local MHA).

Sharding: one attention head per NeuronCore (8 heads, 8 cores), SPMD.
Per core (head h):
  - global branch: bf16 q/k/v projections; softmax(q k^T)*v in "S^T layout"
    (k-tokens on psum partitions, q on free dim). Per batch, the leading
    k-blocks take exp on the Scalar engine emitting fp8(e4m3) P and a PV
    matmul in fp8 DoubleRow mode over k-block PAIRS (256-deep contraction);
    the trailing k-blocks take a one-instruction "fast exp" on the Vector
    engine (P_bits = round_i16(S*A + B) bitcast to bf16, a piecewise-linear
    2^x with ~3% sawtooth that cancels in softmax) and bf16 PV. The split
    ratio differs per batch to balance Scalar/Vector load against the
    interleaved filler work. v carries a ones column at col 64 so the
    softmax denominator falls out of PV.
  - the local branch (windowed attention) and all remaining projections are
    emitted INTERLEAVED into the global loop slots as filler work, so every
    engine stays busy and the PE stays at full clock.
  - normalization on device (reciprocal of the denominator broadcast into
    the psum drain); the local drain also un-permutes windows to image
    order, so g and l are token-aligned.
  - merged projection: g and l proj accumulate into one psum tile; single
    bf16 output.
Host: sums the 8 per-head partials and adds biases.
"""

import sys

sys.path.insert(0, "/opt/trn_rl_repo")

import numpy as np
import ml_dtypes

import concourse.bass as bass
import concourse.mybir as mybir
import concourse.tile as tile
from concourse import bacc, bass_utils

BF16 = mybir.dt.bfloat16
FP8 = mybir.dt.float8e4
I16 = mybir.dt.int16
F32 = mybir.dt.float32
DR = mybir.MatmulPerfMode.DoubleRow
MULT = mybir.AluOpType.mult
ADD = mybir.AluOpType.add
EXP = mybir.ActivationFunctionType.Exp

B, N, C = 2, 3136, 384
H, HD, WS = 8, 48, 7
NT = B * N            # 6272 tokens total
WT = WS * WS          # 49 tokens per window
QB = 448              # q-tile (free dim) for global attention
VS = 65               # v_aug column stride: 48 v + 16 pad + 1 ones
GVS = 80              # fp8 v block stride (16B-aligned for dual-fp8 ldweights)
F8 = (12, 10)         # per-batch count of k-blocks on the fp8/Scalar path
VPK_OFF = (0, F8[0])            # v_pk block offset per batch
VBF_OFF = (0, 25 - F8[0])       # v_bf block offset per batch
NPK = F8[0] + F8[1]             # total fp8 v blocks
NBF = (25 - F8[0]) + (25 - F8[1])  # total bf16 v blocks
SCALE = float(HD) ** -0.5
# fast-exp constants: bf16(bitcast(round_i16(S*FE_A + FE_B))) ~= exp(S*SCALE)
FE_A = SCALE * float(np.log2(np.e)) * 128.0
FE_B = 16250.5


def build_program():
    nc = bacc.Bacc(
        "TRN2",
        target_bir_lowering=False,
        debug=False,
        enable_asserts=False,
        num_devices=8,
    )

    din = {}
    for name, shape in [
        ("xT", (C, NT)), ("winT", (C, NT)),
        ("gwqk", (C, 112)), ("gwv", (C, HD)), ("gwp", (HD, C)),
        ("lwqk", (C, 112)), ("lwv", (C, HD)), ("lwp", (HD, C)),
    ]:
        din[name] = nc.dram_tensor(name, list(shape), BF16, kind="ExternalInput").ap()

    dout = {"out": nc.dram_tensor("out", [NT, C], BF16, kind="ExternalOutput").ap()}

    with tile.TileContext(nc) as tc:
        _emit(tc, nc, din, dout)

    nc.compile()
    return nc


def _emit(tc, nc, din, dout):
    from contextlib import ExitStack

    ctx = ExitStack()
    with ctx:
        persist = ctx.enter_context(tc.tile_pool(name="persist", bufs=1))
        psum = ctx.enter_context(tc.tile_pool(name="psum", bufs=2, space="PSUM"))
        work = ctx.enter_context(tc.tile_pool(name="work", bufs=3))

        # ---- load inputs to SBUF ----
        xt = [persist.tile([128, NT], BF16, name=f"xt{c}") for c in range(3)]
        wt = [persist.tile([128, NT], BF16, name=f"wt{c}") for c in range(3)]
        for c in range(3):
            nc.sync.dma_start(xt[c][:, :], din["xT"][c * 128:(c + 1) * 128, :])
            nc.sync.dma_start(wt[c][:, :], din["winT"][c * 128:(c + 1) * 128, :])
        gwqk = persist.tile([128, 3 * 112], BF16, name="gwqk")
        lwqk = persist.tile([128, 3 * 112], BF16, name="lwqk")
        gwv = persist.tile([128, 3 * 48], BF16, name="gwv")
        lwv = persist.tile([128, 3 * 48], BF16, name="lwv")
        for c in range(3):
            nc.sync.dma_start(gwqk[:, c * 112:(c + 1) * 112], din["gwqk"][c * 128:(c + 1) * 128, :])
            nc.sync.dma_start(lwqk[:, c * 112:(c + 1) * 112], din["lwqk"][c * 128:(c + 1) * 128, :])
            nc.sync.dma_start(gwv[:, c * 48:(c + 1) * 48], din["gwv"][c * 128:(c + 1) * 128, :])
            nc.sync.dma_start(lwv[:, c * 48:(c + 1) * 48], din["lwv"][c * 128:(c + 1) * 128, :])
        gwp = persist.tile([HD, C], BF16, name="gwp")
        lwp = persist.tile([HD, C], BF16, name="lwp")
        nc.sync.dma_start(gwp[:, :], din["gwp"][:, :])
        nc.sync.dma_start(lwp[:, :], din["lwp"][:, :])

        # ---- persistent intermediates ----
        g_qT = persist.tile([HD, NT], BF16, name="g_qT")
        g_kT = persist.tile([HD, NT], BF16, name="g_kT")
        l_qT = persist.tile([HD, NT], BF16, name="l_qT")
        l_kT = persist.tile([HD, NT], BF16, name="l_kT")
        v_pk = persist.tile([128, NPK * GVS], FP8, name="v_pk")
        v_bf = persist.tile([128, NBF * VS], BF16, name="v_bf")
        l_vaug = persist.tile([49, 128 * VS], BF16, name="l_vaug")
        g_outT = persist.tile([HD, NT], BF16, name="g_outT")  # normalized
        l_outT = persist.tile([HD, NT], BF16, name="l_outT")  # normalized, x-order

        v_pk_blk = v_pk[:, :].rearrange("p (k c) -> p k c", c=GVS)
        v_bf_blk = v_bf[:, :].rearrange("p (k c) -> p k c", c=VS)

        # v_pk: zero, ones col 64 (all fp8 blocks are full 128-token blocks)
        nc.gpsimd.memset(v_pk[:, :], 0)
        nc.gpsimd.memset(v_pk_blk[:, :, 64:65], 1.0)
        # v_bf: zero; ones col 64; block 24 of each batch has only 64 rows
        nc.gpsimd.memset(v_bf[:, :], 0)
        for b in range(B):
            i0, i1 = VBF_OFF[b], VBF_OFF[b] + (25 - F8[b])
            nc.gpsimd.memset(v_bf_blk[:, i0:i1 - 1, 64:65], 1.0)
            nc.gpsimd.memset(v_bf_blk[0:64, i1 - 1, 64:65], 1.0)
        nc.gpsimd.memset(l_vaug[:, :].rearrange("p (b k) -> p b k", k=VS)[:, :, 48:VS], 0.0)
        nc.gpsimd.memset(l_vaug[:, :].rearrange("p (b k) -> p b k", k=VS)[:, :, 64:VS], 1.0)

        # ---------- emission helpers ----------
        def qk_proj_tile(br, qb):
            src, qT, kT, wqk = ((xt, g_qT, g_kT, gwqk), (wt, l_qT, l_kT, lwqk))[br]
            t0 = qb * QB
            ps = psum.tile([112, QB], F32, name="pqk", tag="pmix", bufs=2)
            for c in range(3):
                nc.tensor.matmul(ps[:, :], wqk[:, c * 112:(c + 1) * 112],
                                 src[c][:, t0:t0 + QB], start=(c == 0), stop=(c == 2))
            nc.vector.tensor_copy(qT[:, t0:t0 + QB], ps[0:48, :])
            nc.vector.tensor_copy(kT[:, t0:t0 + QB], ps[64:112, :])

        def v_proj_block(b, j):
            sz = 128 if j < 24 else 64
            t0 = b * N + j * 128
            ps = psum.tile([128, HD], F32, name="pv", tag="pmix", bufs=2)
            for c in range(3):
                nc.tensor.matmul(ps[0:sz, :], xt[c][:, t0:t0 + sz],
                                 gwv[:, c * 48:(c + 1) * 48], start=(c == 0), stop=(c == 2))
            if j < F8[b]:
                nc.vector.tensor_copy(v_pk_blk[0:sz, VPK_OFF[b] + j, 0:48], ps[0:sz, :])
            else:
                nc.vector.tensor_copy(v_bf_blk[0:sz, VBF_OFF[b] + j - F8[b], 0:48], ps[0:sz, :])

        def lv_group(g4):
            ps = psum.tile([49, 4 * HD], F32, name="pvl", tag="pmix", bufs=2)
            for u in range(4):
                t0 = (g4 * 4 + u) * WT
                for c in range(3):
                    nc.tensor.matmul(ps[:, u * HD:(u + 1) * HD], wt[c][:, t0:t0 + WT],
                                     lwv[:, c * 48:(c + 1) * 48], start=(c == 0), stop=(c == 2))
            dst = l_vaug[0:WT, (4 * g4) * VS:(4 * g4 + 4) * VS].rearrange(
                "p (u c) -> p u c", c=VS)
            nc.vector.tensor_copy(dst[:, :, 0:48],
                                  ps[:, :].rearrange("p (u c) -> p u c", c=HD))

        def lattn_group(grp):
            b, wh = grp // 8, grp % 8
            psl = psum.tile([49, 392], F32, name="pSl", tag="pmix", bufs=2)
            for w8 in range(8):
                t0 = (grp * 8 + w8) * WT
                nc.tensor.matmul(psl[:, w8 * WT:(w8 + 1) * WT],
                                 l_kT[:, t0:t0 + WT], l_qT[:, t0:t0 + WT],
                                 start=True, stop=True)
            exl16 = work.tile([49, 392], I16, name="expSl")
            nc.vector.tensor_scalar(exl16[:, :], psl[:, :], FE_A, FE_B, MULT, ADD)
            exl = exl16.bitcast(BF16)
            pol = psum.tile([VS, 392], F32, name="poutl", tag="pmix", bufs=2)
            for w8 in range(8):
                w = grp * 8 + w8
                nc.tensor.matmul(pol[:, w8 * WT:(w8 + 1) * WT],
                                 l_vaug[0:WT, w * VS:w * VS + VS],
                                 exl[:, w8 * WT:(w8 + 1) * WT], start=True, stop=True)
            rdl = work.tile([1, 392], F32, name="rdl", tag="dn", bufs=3)
            nc.vector.tensor_copy(rdl[:, :], pol[64:VS, :])
            nc.vector.reciprocal_approx_fast(rdl[:, :], rdl[:, :])
            rbl = work.tile([48, 392], F32, name="rbl", tag="rb", bufs=2)
            nc.gpsimd.partition_broadcast(rbl[:, :], rdl[:, :])
            base = b * N + wh * 392
            dst = l_outT[0:48, base:base + 392].rearrange("p (j w i) -> p w j i", j=7, w=8)
            src = pol[0:48, :].rearrange("p (w j i) -> p w j i", w=8, j=7)
            rb = rbl[0:48, :].rearrange("p (w j i) -> p w j i", w=8, j=7)
            nc.vector.tensor_tensor(dst, src, rb, MULT)

        def proj_block(t0):
            pp = psum.tile([112, C], F32, name="pp", tag="pmix", bufs=2)
            nc.tensor.matmul(pp[:, :], g_outT[:, t0:t0 + 112], gwp[:, :],
                             start=True, stop=False)
            nc.tensor.matmul(pp[:, :], l_outT[:, t0:t0 + 112], lwp[:, :],
                             start=False, stop=True)
            sp = work.tile([112, C], BF16, name="sproj", tag="sproj", bufs=4)
            nc.vector.tensor_copy(sp[:, :], pp[:, :])
            nc.sync.dma_start(dout["out"][t0:t0 + 112, :], sp[:, :])

        # ---------- global attention loop with interleaved fillers ----------
        def global_batch(b, fillers, proj_queue, proj_per_slot):
            nact = F8[b] // 2
            for qc in range(7):
                q0 = b * N + qc * QB
                po = psum.tile([GVS, QB], F32, name="po", tag="po", bufs=2)
                exs = [None] * 13

                def do_pv(s):
                    if s < nact:
                        ex = exs[s]
                        pr0 = (VPK_OFF[b] + 2 * s) * GVS
                        nc.tensor.matmul(po[:, :],
                                         v_pk[:, pr0:pr0 + 2 * GVS].rearrange(
                                             "p (u m) -> p u m", u=2),
                                         ex[:, :].rearrange("p (u t) -> p u t", u=2),
                                         start=(s == 0), stop=False, perf_mode=DR)
                    elif s < 12:
                        exb = exs[s].bitcast(BF16)
                        for u in range(2):
                            blk = 2 * s + u
                            vr0 = (VBF_OFF[b] + blk - F8[b]) * VS
                            nc.tensor.matmul(po[0:VS, :], v_bf[:, vr0:vr0 + VS],
                                             exb[:, u * QB:(u + 1) * QB],
                                             start=False, stop=False)
                    else:
                        exb = exs[s].bitcast(BF16)
                        vr0 = (VBF_OFF[b] + 24 - F8[b]) * VS
                        nc.tensor.matmul(po[0:VS, :], v_bf[0:64, vr0:vr0 + VS],
                                         exb[0:64, :], start=False, stop=True)

                for s in range(13):
                    if s < 12:
                        ps = psum.tile([128, 1024], F32, name="pS", tag="pS", bufs=2)
                        for u in range(2):
                            k0 = b * N + (2 * s + u) * 128
                            nc.tensor.matmul(ps[:, u * 512:u * 512 + QB],
                                             g_kT[:, k0:k0 + 128],
                                             g_qT[:, q0:q0 + QB],
                                             start=True, stop=True)
                        ps_v = ps[:, :].rearrange("p (u k) -> p u k", k=512)[:, :, 0:QB]
                        if s < nact:
                            ex = work.tile([128, 2 * QB], FP8, name="expS")
                            nc.scalar.activation(
                                ex[:, :].rearrange("p (u k) -> p u k", k=QB), ps_v,
                                EXP, scale=SCALE)
                            exs[s] = ex
                        else:
                            exb = work.tile([128, 2 * QB], I16, name="expB", tag="expB", bufs=3)
                            nc.vector.tensor_scalar(
                                exb[:, :].rearrange("p (u k) -> p u k", k=QB), ps_v,
                                FE_A, FE_B, MULT, ADD)
                            exs[s] = exb
                    else:
                        ps = psum.tile([128, 1024], F32, name="pS", tag="pS", bufs=2)
                        k0 = b * N + 24 * 128
                        nc.tensor.matmul(ps[0:64, 0:QB], g_kT[:, k0:k0 + 64],
                                         g_qT[:, q0:q0 + QB], start=True, stop=True)
                        exb = work.tile([64, QB], I16, name="expB24", tag="expB24", bufs=2)
                        nc.vector.tensor_scalar(exb[:, :], ps[0:64, 0:QB],
                                                FE_A, FE_B, MULT, ADD)
                        exs[s] = exb
                    if s >= 1:
                        do_pv(s - 1)
                    if fillers:
                        fillers.pop(0)()
                    for _ in range(proj_per_slot):
                        if proj_queue:
                            proj_block(proj_queue.pop(0))
                do_pv(12)

                rd = work.tile([1, QB], F32, name="rd", tag="dn", bufs=3)
                nc.vector.tensor_copy(rd[:, :], po[64:VS, :])
                nc.vector.reciprocal_approx_fast(rd[:, :], rd[:, :])
                rbg = work.tile([48, QB], F32, name="rbg", tag="rb", bufs=2)
                nc.gpsimd.partition_broadcast(rbg[:, :], rd[:, :])
                nc.vector.tensor_tensor(g_outT[:, q0:q0 + QB], po[0:48, :],
                                        rbg[0:48, :], MULT)
                if b == 1:
                    for blk in range(4):
                        proj_queue.append(q0 + blk * 112)

        # prologue: only what batch-0 global needs (its k/q tiles + v blocks)
        for qb in range(7):
            qk_proj_tile(0, qb)
        for j in range(25):
            v_proj_block(0, j)

        # batch-0 global loop; fillers = batch-1 qk/v proj, local qk proj,
        # local v, local attention (in dependency-friendly order)
        fillers = []
        for qb in range(7, 14):
            fillers.append(lambda qb=qb: qk_proj_tile(0, qb))
        for j in range(25):
            fillers.append(lambda j=j: v_proj_block(1, j))
        for qb in range(14):
            fillers.append(lambda qb=qb: qk_proj_tile(1, qb))
        for g4 in range(32):
            fillers.append(lambda g4=g4: lv_group(g4))
        for grp in range(16):
            fillers.append(lambda grp=grp: lattn_group(grp))
        global_batch(0, fillers, [], 0)
        while fillers:
            fillers.pop(0)()

        # batch-1 global loop; fillers = projection of batch-0 tokens and
        # (as they complete) batch-1 tokens
        proj_queue = [t0 for t0 in range(0, N, 112)]
        global_batch(1, [], proj_queue, 1)
        while proj_queue:
            proj_block(proj_queue.pop(0))


def _host_prep(x, g_qkv_w, g_proj_w, l_qkv_w, l_proj_w):
    bf = ml_dtypes.bfloat16
    xf = np.asarray(x, np.float32).reshape(NT, C)
    xT = np.ascontiguousarray(xf.T).astype(bf)
    x4 = np.asarray(x, np.float32).reshape(B, 56, 56, C)
    win = x4.reshape(B, 8, WS, 8, WS, C).transpose(0, 1, 3, 5, 2, 4)
    win = win.reshape(B, 8, 8, WS, WS, C).transpose(0, 1, 2, 4, 3, 5).reshape(NT, C)
    winT = np.ascontiguousarray(win.T).astype(bf)

    in_maps = []
    for h in range(8):
        m = {"xT": xT, "winT": winT}
        for pre, qkv_w, proj_w in (("g", g_qkv_w, g_proj_w), ("l", l_qkv_w, l_proj_w)):
            qw = np.asarray(qkv_w[:, h * HD:(h + 1) * HD], np.float32)
            kw = np.asarray(qkv_w[:, C + h * HD:C + (h + 1) * HD], np.float32)
            vw = np.asarray(qkv_w[:, 2 * C + h * HD:2 * C + (h + 1) * HD], np.float32)
            wqk = np.zeros((C, 112), np.float32)
            wqk[:, 0:48] = qw
            wqk[:, 64:112] = kw
            m[pre + "wqk"] = wqk.astype(bf)
            m[pre + "wv"] = np.ascontiguousarray(vw).astype(bf)
            m[pre + "wp"] = np.ascontiguousarray(
                np.asarray(proj_w, np.float32)[h * HD:(h + 1) * HD, :]).astype(bf)
        in_maps.append(m)
    return in_maps


_NC_CACHE = None


def kernel(x, g_qkv_w, g_proj_w, g_proj_b, l_qkv_w, l_proj_w, l_proj_b):
    global _NC_CACHE
    if _NC_CACHE is None:
        _NC_CACHE = build_program()
    nc = _NC_CACHE

    in_maps = _host_prep(x, g_qkv_w, g_proj_w, l_qkv_w, l_proj_w)
    res = bass_utils.run_bass_kernel_spmd(nc, in_maps, core_ids=list(range(8)))

    acc = np.zeros((NT, C), np.float32)
    for h in range(8):
        acc += np.asarray(res.results[h]["out"], np.float32)
    out = acc + np.asarray(g_proj_b, np.float32) + np.asarray(l_proj_b, np.float32)
    return out.reshape(B, N, C).astype(np.float32)


# revision 22
# speedup vs baseline: 1.1051x; 1.1051x over previous
"""Trainium2 Bass kernel for LGAttention (global MHA + windowed local MHA).

Sharding: one attention head per NeuronCore (8 heads, 8 cores), SPMD.
Per core (head h):
  - global branch: bf16 q/k/v projections; softmax(q k^T)*v in "S^T layout"
    (k-tokens on psum partitions, q on free dim). Per batch, the leading
    k-blocks take exp on the Scalar engine emitting fp8(e4m3) P and a PV
    matmul in fp8 DoubleRow mode over k-block PAIRS (256-deep contraction);
    the trailing k-blocks take a one-instruction "fast exp" on the Vector
    engine (P_bits = round_i16(S*A + B) bitcast to bf16, a piecewise-linear
    2^x with ~3% sawtooth that cancels in softmax) and bf16 PV. The split
    ratio differs per batch to balance Scalar/Vector load against the
    interleaved filler work. v carries a ones column at col 64 so the
    softmax denominator falls out of PV.
  - the local branch (windowed attention) and all remaining projections are
    emitted INTERLEAVED into the global loop slots as filler work, so every
    engine stays busy and the PE stays at full clock.
  - normalization on device (reciprocal of the denominator broadcast into
    the psum drain); the local drain also un-permutes windows to image
    order, so g and l are token-aligned.
  - merged projection: g and l proj accumulate into one psum tile; single
    bf16 output.
Host: sums the 8 per-head partials and adds biases.
"""

import sys

sys.path.insert(0, "/opt/trn_rl_repo")

import numpy as np
import ml_dtypes

import concourse.bass as bass
import concourse.mybir as mybir
import concourse.tile as tile
from concourse import bacc, bass_utils

BF16 = mybir.dt.bfloat16
FP8 = mybir.dt.float8e4
I16 = mybir.dt.int16
F32 = mybir.dt.float32
DR = mybir.MatmulPerfMode.DoubleRow
MULT = mybir.AluOpType.mult
ADD = mybir.AluOpType.add
EXP = mybir.ActivationFunctionType.Exp

B, N, C = 2, 3136, 384
H, HD, WS = 8, 48, 7
NT = B * N            # 6272 tokens total
WT = WS * WS          # 49 tokens per window
QB = 448              # q-tile (free dim) for global attention
VS = 65               # v_aug column stride: 48 v + 16 pad + 1 ones
GVS = 80              # fp8 v block stride (16B-aligned for dual-fp8 ldweights)
F8 = (20, 14)         # per-batch count of k-blocks on the fp8/Scalar path
VPK_OFF = (0, F8[0])            # v_pk block offset per batch
VBF_OFF = (0, 25 - F8[0])       # v_bf block offset per batch
NPK = F8[0] + F8[1]             # total fp8 v blocks
NBF = (25 - F8[0]) + (25 - F8[1])  # total bf16 v blocks
SCALE = float(HD) ** -0.5
# fast-exp constants: bf16(bitcast(round_i16(S*FE_A + FE_B))) ~= exp(S*SCALE)
FE_A = SCALE * float(np.log2(np.e)) * 128.0
FE_B = 16250.5


def build_program():
    nc = bacc.Bacc(
        "TRN2",
        target_bir_lowering=False,
        debug=False,
        enable_asserts=False,
        num_devices=8,
    )

    din = {}
    for name, shape in [
        ("xT", (C, NT)), ("winT", (C, NT)),
        ("gwqk", (C, 112)), ("gwv", (C, HD)), ("gwp", (HD, C)),
        ("lwqk", (C, 112)), ("lwv", (C, HD)), ("lwp", (HD, C)),
    ]:
        din[name] = nc.dram_tensor(name, list(shape), BF16, kind="ExternalInput").ap()

    dout = {"out": nc.dram_tensor("out", [NT, C], BF16, kind="ExternalOutput").ap()}

    with tile.TileContext(nc) as tc:
        _emit(tc, nc, din, dout)

    nc.compile()
    return nc


def _emit(tc, nc, din, dout):
    from contextlib import ExitStack

    ctx = ExitStack()
    with ctx:
        persist = ctx.enter_context(tc.tile_pool(name="persist", bufs=1))
        psum = ctx.enter_context(tc.tile_pool(name="psum", bufs=2, space="PSUM"))
        work = ctx.enter_context(tc.tile_pool(name="work", bufs=3))

        # ---- load inputs to SBUF ----
        xt = [persist.tile([128, NT], BF16, name=f"xt{c}") for c in range(3)]
        wt = [persist.tile([128, NT], BF16, name=f"wt{c}") for c in range(3)]
        for c in range(3):
            nc.sync.dma_start(xt[c][:, :], din["xT"][c * 128:(c + 1) * 128, :])
            nc.sync.dma_start(wt[c][:, :], din["winT"][c * 128:(c + 1) * 128, :])
        gwqk = persist.tile([128, 3 * 112], BF16, name="gwqk")
        lwqk = persist.tile([128, 3 * 112], BF16, name="lwqk")
        gwv = persist.tile([128, 3 * 48], BF16, name="gwv")
        lwv = persist.tile([128, 3 * 48], BF16, name="lwv")
        for c in range(3):
            nc.sync.dma_start(gwqk[:, c * 112:(c + 1) * 112], din["gwqk"][c * 128:(c + 1) * 128, :])
            nc.sync.dma_start(lwqk[:, c * 112:(c + 1) * 112], din["lwqk"][c * 128:(c + 1) * 128, :])
            nc.sync.dma_start(gwv[:, c * 48:(c + 1) * 48], din["gwv"][c * 128:(c + 1) * 128, :])
            nc.sync.dma_start(lwv[:, c * 48:(c + 1) * 48], din["lwv"][c * 128:(c + 1) * 128, :])
        gwp = persist.tile([HD, C], BF16, name="gwp")
        lwp = persist.tile([HD, C], BF16, name="lwp")
        nc.sync.dma_start(gwp[:, :], din["gwp"][:, :])
        nc.sync.dma_start(lwp[:, :], din["lwp"][:, :])

        # ---- persistent intermediates ----
        g_qT = persist.tile([HD, NT], BF16, name="g_qT")
        g_kT = persist.tile([HD, NT], BF16, name="g_kT")
        l_qT = persist.tile([HD, NT], BF16, name="l_qT")
        l_kT = persist.tile([HD, NT], BF16, name="l_kT")
        v_pk = persist.tile([128, NPK * GVS], FP8, name="v_pk")
        v_bf = persist.tile([128, NBF * VS], BF16, name="v_bf")
        l_vaug = persist.tile([49, 128 * VS], BF16, name="l_vaug")
        g_outT = persist.tile([HD, NT], BF16, name="g_outT")  # normalized
        l_outT = persist.tile([HD, NT], BF16, name="l_outT")  # normalized, x-order

        v_pk_blk = v_pk[:, :].rearrange("p (k c) -> p k c", c=GVS)
        v_bf_blk = v_bf[:, :].rearrange("p (k c) -> p k c", c=VS)

        # v_pk: zero, ones col 64 (all fp8 blocks are full 128-token blocks)
        nc.gpsimd.memset(v_pk[:, :], 0)
        nc.gpsimd.memset(v_pk_blk[:, :, 64:65], 1.0)
        # v_bf: zero; ones col 64; block 24 of each batch has only 64 rows
        nc.gpsimd.memset(v_bf[:, :], 0)
        for b in range(B):
            i0, i1 = VBF_OFF[b], VBF_OFF[b] + (25 - F8[b])
            nc.gpsimd.memset(v_bf_blk[:, i0:i1 - 1, 64:65], 1.0)
            nc.gpsimd.memset(v_bf_blk[0:64, i1 - 1, 64:65], 1.0)
        nc.gpsimd.memset(l_vaug[:, :].rearrange("p (b k) -> p b k", k=VS)[:, :, 48:VS], 0.0)
        nc.gpsimd.memset(l_vaug[:, :].rearrange("p (b k) -> p b k", k=VS)[:, :, 64:VS], 1.0)

        # ---------- emission helpers ----------
        def qk_proj_tile(br, qb):
            src, qT, kT, wqk = ((xt, g_qT, g_kT, gwqk), (wt, l_qT, l_kT, lwqk))[br]
            t0 = qb * QB
            ps = psum.tile([112, QB], F32, name="pqk", tag="pmix", bufs=2)
            for c in range(3):
                nc.tensor.matmul(ps[:, :], wqk[:, c * 112:(c + 1) * 112],
                                 src[c][:, t0:t0 + QB], start=(c == 0), stop=(c == 2))
            nc.vector.tensor_copy(qT[:, t0:t0 + QB], ps[0:48, :])
            nc.vector.tensor_copy(kT[:, t0:t0 + QB], ps[64:112, :])

        def v_proj_block(b, j):
            sz = 128 if j < 24 else 64
            t0 = b * N + j * 128
            ps = psum.tile([128, HD], F32, name="pv", tag="pmix", bufs=2)
            for c in range(3):
                nc.tensor.matmul(ps[0:sz, :], xt[c][:, t0:t0 + sz],
                                 gwv[:, c * 48:(c + 1) * 48], start=(c == 0), stop=(c == 2))
            if j < F8[b]:
                nc.vector.tensor_copy(v_pk_blk[0:sz, VPK_OFF[b] + j, 0:48], ps[0:sz, :])
            else:
                nc.vector.tensor_copy(v_bf_blk[0:sz, VBF_OFF[b] + j - F8[b], 0:48], ps[0:sz, :])

        def lv_group(g4):
            ps = psum.tile([49, 4 * HD], F32, name="pvl", tag="pmix", bufs=2)
            for u in range(4):
                t0 = (g4 * 4 + u) * WT
                for c in range(3):
                    nc.tensor.matmul(ps[:, u * HD:(u + 1) * HD], wt[c][:, t0:t0 + WT],
                                     lwv[:, c * 48:(c + 1) * 48], start=(c == 0), stop=(c == 2))
            dst = l_vaug[0:WT, (4 * g4) * VS:(4 * g4 + 4) * VS].rearrange(
                "p (u c) -> p u c", c=VS)
            nc.vector.tensor_copy(dst[:, :, 0:48],
                                  ps[:, :].rearrange("p (u c) -> p u c", c=HD))

        def lattn_group(grp):
            b, wh = grp // 8, grp % 8
            psl = psum.tile([49, 392], F32, name="pSl", tag="pmix", bufs=2)
            for w8 in range(8):
                t0 = (grp * 8 + w8) * WT
                nc.tensor.matmul(psl[:, w8 * WT:(w8 + 1) * WT],
                                 l_kT[:, t0:t0 + WT], l_qT[:, t0:t0 + WT],
                                 start=True, stop=True)
            exl = work.tile([49, 392], BF16, name="expSl")
            nc.scalar.activation(exl[:, :], psl[:, :], EXP, scale=SCALE)
            pol = psum.tile([VS, 392], F32, name="poutl", tag="pmix", bufs=2)
            for w8 in range(8):
                w = grp * 8 + w8
                nc.tensor.matmul(pol[:, w8 * WT:(w8 + 1) * WT],
                                 l_vaug[0:WT, w * VS:w * VS + VS],
                                 exl[:, w8 * WT:(w8 + 1) * WT], start=True, stop=True)
            rdl = work.tile([1, 392], F32, name="rdl", tag="dn", bufs=3)
            nc.vector.tensor_copy(rdl[:, :], pol[64:VS, :])
            nc.vector.reciprocal_approx_fast(rdl[:, :], rdl[:, :])
            rbl = work.tile([48, 392], F32, name="rbl", tag="rb", bufs=2)
            nc.gpsimd.partition_broadcast(rbl[:, :], rdl[:, :])
            base = b * N + wh * 392
            dst = l_outT[0:48, base:base + 392].rearrange("p (j w i) -> p w j i", j=7, w=8)
            src = pol[0:48, :].rearrange("p (w j i) -> p w j i", w=8, j=7)
            rb = rbl[0:48, :].rearrange("p (w j i) -> p w j i", w=8, j=7)
            nc.vector.tensor_tensor(dst, src, rb, MULT)

        def proj_block(t0):
            pp = psum.tile([112, C], F32, name="pp", tag="pmix", bufs=2)
            nc.tensor.matmul(pp[:, :], g_outT[:, t0:t0 + 112], gwp[:, :],
                             start=True, stop=False)
            nc.tensor.matmul(pp[:, :], l_outT[:, t0:t0 + 112], lwp[:, :],
                             start=False, stop=True)
            sp = work.tile([112, C], BF16, name="sproj", tag="sproj", bufs=4)
            nc.vector.tensor_copy(sp[:, :], pp[:, :])
            nc.sync.dma_start(dout["out"][t0:t0 + 112, :], sp[:, :])

        # ---------- global attention loop with interleaved fillers ----------
        def global_batch(b, fillers, proj_queue, proj_per_slot):
            nact = F8[b] // 2
            for qc in range(7):
                q0 = b * N + qc * QB
                po = psum.tile([GVS, QB], F32, name="po", tag="po", bufs=2)
                exs = [None] * 13

                def do_pv(s):
                    if s < nact:
                        ex = exs[s]
                        pr0 = (VPK_OFF[b] + 2 * s) * GVS
                        nc.tensor.matmul(po[:, :],
                                         v_pk[:, pr0:pr0 + 2 * GVS].rearrange(
                                             "p (u m) -> p u m", u=2),
                                         ex[:, :].rearrange("p (u t) -> p u t", u=2),
                                         start=(s == 0), stop=False, perf_mode=DR)
                    elif s < 12:
                        exb = exs[s].bitcast(BF16)
                        for u in range(2):
                            blk = 2 * s + u
                            vr0 = (VBF_OFF[b] + blk - F8[b]) * VS
                            nc.tensor.matmul(po[0:VS, :], v_bf[:, vr0:vr0 + VS],
                                             exb[:, u * QB:(u + 1) * QB],
                                             start=False, stop=False)
                    else:
                        exb = exs[s].bitcast(BF16)
                        vr0 = (VBF_OFF[b] + 24 - F8[b]) * VS
                        nc.tensor.matmul(po[0:VS, :], v_bf[0:64, vr0:vr0 + VS],
                                         exb[0:64, :], start=False, stop=True)

                for s in range(13):
                    if s < 12:
                        ps = psum.tile([128, 1024], F32, name="pS", tag="pS", bufs=2)
                        for u in range(2):
                            k0 = b * N + (2 * s + u) * 128
                            nc.tensor.matmul(ps[:, u * 512:u * 512 + QB],
                                             g_kT[:, k0:k0 + 128],
                                             g_qT[:, q0:q0 + QB],
                                             start=True, stop=True)
                        ps_v = ps[:, :].rearrange("p (u k) -> p u k", k=512)[:, :, 0:QB]
                        if s < nact:
                            ex = work.tile([128, 2 * QB], FP8, name="expS")
                            nc.scalar.activation(
                                ex[:, :].rearrange("p (u k) -> p u k", k=QB), ps_v,
                                EXP, scale=SCALE)
                            exs[s] = ex
                        else:
                            exb = work.tile([128, 2 * QB], I16, name="expB", tag="expB", bufs=3)
                            nc.vector.tensor_scalar(
                                exb[:, :].rearrange("p (u k) -> p u k", k=QB), ps_v,
                                FE_A, FE_B, MULT, ADD)
                            exs[s] = exb
                    else:
                        ps = psum.tile([128, 1024], F32, name="pS", tag="pS", bufs=2)
                        k0 = b * N + 24 * 128
                        nc.tensor.matmul(ps[0:64, 0:QB], g_kT[:, k0:k0 + 64],
                                         g_qT[:, q0:q0 + QB], start=True, stop=True)
                        exb = work.tile([64, QB], I16, name="expB24", tag="expB24", bufs=2)
                        nc.vector.tensor_scalar(exb[:, :], ps[0:64, 0:QB],
                                                FE_A, FE_B, MULT, ADD)
                        exs[s] = exb
                    if s >= 1:
                        do_pv(s - 1)
                    if fillers:
                        fillers.pop(0)()
                    for _ in range(proj_per_slot):
                        if proj_queue:
                            proj_block(proj_queue.pop(0))
                do_pv(12)

                rd = work.tile([1, QB], F32, name="rd", tag="dn", bufs=3)
                nc.vector.tensor_copy(rd[:, :], po[64:VS, :])
                nc.vector.reciprocal_approx_fast(rd[:, :], rd[:, :])
                rbg = work.tile([48, QB], F32, name="rbg", tag="rb", bufs=2)
                nc.gpsimd.partition_broadcast(rbg[:, :], rd[:, :])
                nc.vector.tensor_tensor(g_outT[:, q0:q0 + QB], po[0:48, :],
                                        rbg[0:48, :], MULT)
                if b == 1:
                    for blk in range(4):
                        proj_queue.append(q0 + blk * 112)

        # prologue: only what batch-0 global needs (its k/q tiles + v blocks)
        for qb in range(7):
            qk_proj_tile(0, qb)
        for j in range(25):
            v_proj_block(0, j)

        # batch-0 global loop; fillers = batch-1 qk/v proj, local qk proj,
        # local v, local attention (in dependency-friendly order)
        fillers = []
        for qb in range(7, 14):
            fillers.append(lambda qb=qb: qk_proj_tile(0, qb))
        for j in range(25):
            fillers.append(lambda j=j: v_proj_block(1, j))
        for qb in range(14):
            fillers.append(lambda qb=qb: qk_proj_tile(1, qb))
        for g4 in range(32):
            fillers.append(lambda g4=g4: lv_group(g4))
        for grp in range(16):
            fillers.append(lambda grp=grp: lattn_group(grp))
        global_batch(0, fillers, [], 0)
        while fillers:
            fillers.pop(0)()

        # batch-1 global loop; fillers = projection of batch-0 tokens and
        # (as they complete) batch-1 tokens
        proj_queue = [t0 for t0 in range(0, N, 112)]
        global_batch(1, [], proj_queue, 1)
        while proj_queue:
            proj_block(proj_queue.pop(0))


def _host_prep(x, g_qkv_w, g_proj_w, l_qkv_w, l_proj_w):
    bf = ml_dtypes.bfloat16
    xf = np.asarray(x, np.float32).reshape(NT, C)
    xT = np.ascontiguousarray(xf.T).astype(bf)
    x4 = np.asarray(x, np.float32).reshape(B, 56, 56, C)
    win = x4.reshape(B, 8, WS, 8, WS, C).transpose(0, 1, 3, 5, 2, 4)
    win = win.reshape(B, 8, 8, WS, WS, C).transpose(0, 1, 2, 4, 3, 5).reshape(NT, C)
    winT = np.ascontiguousarray(win.T).astype(bf)

    in_maps = []
    for h in range(8):
        m = {"xT": xT, "winT": winT}
        for pre, qkv_w, proj_w in (("g", g_qkv_w, g_proj_w), ("l", l_qkv_w, l_proj_w)):
            qw = np.asarray(qkv_w[:, h * HD:(h + 1) * HD], np.float32)
            kw = np.asarray(qkv_w[:, C + h * HD:C + (h + 1) * HD], np.float32)
            vw = np.asarray(qkv_w[:, 2 * C + h * HD:2 * C + (h + 1) * HD], np.float32)
            wqk = np.zeros((C, 112), np.float32)
            wqk[:, 0:48] = qw
            wqk[:, 64:112] = kw
            m[pre + "wqk"] = wqk.astype(bf)
            m[pre + "wv"] = np.ascontiguousarray(vw).astype(bf)
            m[pre + "wp"] = np.ascontiguousarray(
                np.asarray(proj_w, np.float32)[h * HD:(h + 1) * HD, :]).astype(bf)
        in_maps.append(m)
    return in_maps


_NC_CACHE = None


def kernel(x, g_qkv_w, g_proj_w, g_proj_b, l_qkv_w, l_proj_w, l_proj_b):
    global _NC_CACHE
    if _NC_CACHE is None:
        _NC_CACHE = build_program()
    nc = _NC_CACHE

    in_maps = _host_prep(x, g_qkv_w, g_proj_w, l_qkv_w, l_proj_w)
    res = bass_utils.run_bass_kernel_spmd(nc, in_maps, core_ids=list(range(8)))

    acc = np.zeros((NT, C), np.float32)
    for h in range(8):
        acc += np.asarray(res.results[h]["out"], np.float32)
    out = acc + np.asarray(g_proj_b, np.float32) + np.asarray(l_proj_b, np.float32)
    return out.reshape(B, N, C).astype(np.float32)
